# revision 10
# baseline (speedup 1.0000x reference)
"""Bidirectional Mamba (BiMambaWrapper) Trainium2 kernel.

Sharding: 8 cores = (batch 0..3) x (direction fw/bw). One uniform SPMD
program; all direction asymmetry is data-driven:
  - bw cores receive host-time-flipped x[b] and the bw weight set.
  - The final 1x1 conv is pre-multiplied into per-direction matrices
    M_f = conv_w[:, :512, 0] @ fw_Wout, M_b = conv_w[:, 512:, 0] @ bw_Wout;
    each core accumulates P = M_own @ g_own + M_oth @ g_oth in PSUM.
  - Each core sends g (no time flip: the reference concatenates the\n    backward branch in its reversed time order) via per-d-block
    AllGather; every core reads gather slot 1, which is the partner for
    even (fw) cores. Odd cores produce garbage epilogues the host drops.
  - Epilogue (bias, GLU, GroupNorm) runs redundantly per pair; the host
    takes even cores' outputs.

Per core: D=1024 d_inner (8 blocks of 128 partitions), T=2048, N=16.
Selective scan via DVE tensor_tensor_scan per (block, n) over [128, 2048].
"""
import sys
sys.path.insert(0, "/opt/trn_rl_repo")

from contextlib import ExitStack

import numpy as np
import ml_dtypes

import concourse.bass as bass
import concourse.mybir as mybir
import concourse.tile as tile
from concourse import bacc
from concourse.bass import ts
from concourse.bass_utils import run_bass_kernel_spmd

F32 = mybir.dt.float32
BF16 = mybir.dt.bfloat16
AF = mybir.ActivationFunctionType
OP = mybir.AluOpType

C = 512          # d_model
T = 2048
D = 1024         # d_inner
N = 16           # d_state
R = 32           # dt_rank
DB = D // 128    # 8 d blocks
CB = C // 128    # 4 c blocks
TF = T // 512    # 4 free chunks for matmuls
EPS = 1e-5

# how many of the 44 movable bf16 TT ops per j go to DVE (rest on Pool)
_N_DVE_XN = {0, 2, 4, 6, 8, 10, 12, 14, 1, 3, 5}   # xn ops on DVE for these n
_N_DVE_P = {0, 2, 4, 6, 8, 10}                      # p ops on DVE for these n

_CACHE = {}


def _rev(ap):
    """Reverse the innermost free dim of a 2-D [partition, free] AP."""
    (pstep, pcount), (fstep, fcount) = ap.ap[0], ap.ap[1]
    return bass.AP(
        tensor=ap.tensor,
        offset=ap.offset + (fcount - 1) * fstep,
        ap=[[pstep, pcount], [-fstep, fcount]],
    )


def _bcast_row(dram_ap, parts=128):
    """Broadcast a 1-D DRAM row AP across `parts` partitions."""
    return bass.AP(tensor=dram_ap.tensor, offset=dram_ap.offset,
                   ap=[[0, parts]] + [list(d) for d in dram_ap.ap])


def build_program():
    nc = bacc.Bacc("TRN2", target_bir_lowering=False)

    x_in = nc.declare_dram_parameter("x", [C, T], F32, isOutput=False)
    winT = nc.declare_dram_parameter("winT", [C, 2 * D], F32, isOutput=False)
    wxT = nc.declare_dram_parameter("wxT", [D, 64], BF16, isOutput=False)
    wdtT = nc.declare_dram_parameter("wdtT", [R, D], F32, isOutput=False)
    mownT = nc.declare_dram_parameter("mownT", [D, D], BF16, isOutput=False)
    mothT = nc.declare_dram_parameter("mothT", [D, D], BF16, isOutput=False)
    a_sc = nc.declare_dram_parameter("a_sc", [128, DB, N], F32, isOutput=False)
    taps = nc.declare_dram_parameter("taps", [128, DB, 4], F32, isOutput=False)
    convb = nc.declare_dram_parameter("convb", [128, DB], F32, isOutput=False)
    bdt = nc.declare_dram_parameter("bdt", [128, DB], F32, isOutput=False)
    dv = nc.declare_dram_parameter("dv", [128, DB], F32, isOutput=False)
    cvb = nc.declare_dram_parameter("cvb", [128, DB], F32, isOutput=False)
    gnw = nc.declare_dram_parameter("gnw", [128, CB], F32, isOutput=False)
    gnb = nc.declare_dram_parameter("gnb", [128, CB], F32, isOutput=False)
    out = nc.declare_dram_parameter("out", [C, T], F32, isOutput=True)

    with tile.TileContext(nc) as tc, ExitStack() as ctx:
        dram = ctx.enter_context(tc.tile_pool(name="dram", bufs=1, space="DRAM"))
        u_dram = dram.tile([128, DB, T], BF16)
        dt_dram = dram.tile([128, DB, T], BF16)
        sz_dram = dram.tile([128, DB, T], BF16)
        s_dram = dram.tile([128, DB, T], F32)
        bc_dram = dram.tile([2 * N, T], BF16)
        st_dram = dram.tile([1, 2], F32)
        gsend = [dram.tile([128, T], BF16, tag=f"gs{j}", name=f"gs{j}") for j in range(DB)]
        ggath = [dram.tile([2, 128, T], BF16, tag=f"gg{j}", name=f"gg{j}")
                 for j in range(DB)]

        consts = ctx.enter_context(tc.tile_pool(name="consts", bufs=1))
        a_t = consts.tile([128, DB, N], F32)
        taps_t = consts.tile([128, DB, 4], F32)
        convb_t = consts.tile([128, DB], F32)
        bdt_t = consts.tile([128, DB], F32)
        dv_t = consts.tile([128, DB], F32)
        cvb_t = consts.tile([128, DB], F32)
        gnw_t = consts.tile([128, CB], F32)
        gnb_t = consts.tile([128, CB], F32)
        ones_t = consts.tile([128, 1], F32)
        eps_t = consts.tile([128, 1], F32)
        nc.vector.memset(eps_t, EPS)
        for t_, s_ in ((a_t, a_sc), (taps_t, taps), (convb_t, convb),
                       (bdt_t, bdt), (dv_t, dv), (cvb_t, cvb),
                       (gnw_t, gnw), (gnb_t, gnb)):
            nc.sync.dma_start(out=t_, in_=s_[:])
        nc.vector.memset(ones_t, 1.0)

        persist = ctx.enter_context(tc.tile_pool(name="persist", bufs=1))
        g_t = persist.tile([128, DB, T], BF16)      # own g (own-time)

        # ================= Phase 1: Win matmul, conv, silu =================
        with tc.tile_pool(name="s1", bufs=2) as s1, \
             tc.tile_pool(name="s1x", bufs=1) as s1x, \
             tc.tile_pool(name="ps1", bufs=2, space="PSUM") as ps1:
            x_sb = s1x.tile([128, CB, T], F32)
            for cb in range(CB):
                nc.sync.dma_start(out=x_sb[:, cb, :], in_=x_in[ts(cb, 128), :])
            for j in range(DB):
                # xc block j: lhsT = winT[:, j*128:(j+1)*128]
                lws = []
                for cb in range(CB):
                    lw = s1.tile([128, 128], F32, tag=f"lw{cb}")
                    nc.sync.dma_start(out=lw, in_=winT[ts(cb, 128), ts(j, 128)])
                    lws.append(lw)
                xc = s1.tile([128, 3 + T], F32, tag="xc")
                nc.vector.memset(xc[:, 0:3], 0.0)
                for tf in range(TF):
                    ps = ps1.tile([128, 512], F32, tag="ps_xc")
                    for cb in range(CB):
                        nc.tensor.matmul(out=ps, lhsT=lws[cb],
                                         rhs=x_sb[:, cb, ts(tf, 512)],
                                         start=(cb == 0), stop=(cb == CB - 1))
                    nc.scalar.copy(out=xc[:, 3 + tf * 512: 3 + (tf + 1) * 512], in_=ps)
                # depthwise causal conv: out[t] = sum_k taps[k]*xc[t-3+k] + convb
                cv = s1.tile([128, T], F32, tag="cv")
                t0 = s1.tile([128, T], F32, tag="t0")
                nc.vector.tensor_scalar(out=t0, in0=xc[:, 0:T],
                                        scalar1=taps_t[:, j, 0:1], scalar2=None,
                                        op0=OP.mult)
                nc.vector.scalar_tensor_tensor(out=cv, in0=xc[:, 1:1 + T],
                                               scalar=taps_t[:, j, 1:2], in1=t0,
                                               op0=OP.mult, op1=OP.add)
                nc.vector.scalar_tensor_tensor(out=t0, in0=xc[:, 2:2 + T],
                                               scalar=taps_t[:, j, 2:3], in1=cv,
                                               op0=OP.mult, op1=OP.add)
                nc.vector.scalar_tensor_tensor(out=cv, in0=xc[:, 3:3 + T],
                                               scalar=taps_t[:, j, 3:4], in1=t0,
                                               op0=OP.mult, op1=OP.add)
                u_bf = s1.tile([128, T], BF16, tag="u_bf")
                nc.scalar.activation(out=u_bf, in_=cv, func=AF.Silu,
                                     bias=convb_t[:, j:j + 1], scale=1.0)
                nc.sync.dma_start(out=u_dram[:, j, :], in_=u_bf)
                # z block j: Win rows D + j*128 -> silu -> spill
                lwzs = []
                for cb in range(CB):
                    lwz = s1.tile([128, 128], F32, tag=f"lwz{cb}")
                    nc.sync.dma_start(out=lwz,
                                      in_=winT[ts(cb, 128), D + j * 128: D + (j + 1) * 128])
                    lwzs.append(lwz)
                for tf in range(TF):
                    ps = ps1.tile([128, 512], F32, tag="ps_z")
                    for cb in range(CB):
                        nc.tensor.matmul(out=ps, lhsT=lwzs[cb],
                                         rhs=x_sb[:, cb, ts(tf, 512)],
                                         start=(cb == 0), stop=(cb == CB - 1))
                    szt = s1.tile([128, 512], BF16, tag="szt")
                    nc.scalar.activation(out=szt, in_=ps, func=AF.Silu, scale=1.0)
                    nc.sync.dma_start(out=sz_dram[:, j, ts(tf, 512)], in_=szt)

        # ================= Phase 2: x_dbl, dt, B/C bounce =================
        with tc.tile_pool(name="s2", bufs=2) as s2, \
             tc.tile_pool(name="s2w", bufs=1) as s2w, \
             tc.tile_pool(name="ps2", bufs=2, space="PSUM") as ps2:
            wx_all = s2w.tile([128, DB, 64], BF16)
            for j in range(DB):
                nc.sync.dma_start(out=wx_all[:, j, :], in_=wxT[ts(j, 128), :])
            xdbl_t = s2w.tile([64, T], F32)
            for tf in range(TF):
                ps = ps2.tile([64, 512], F32, tag="ps_xd")
                for j in range(DB):
                    u_j = s2.tile([128, 512], BF16, tag="u_s")
                    nc.sync.dma_start(out=u_j, in_=u_dram[:, j, ts(tf, 512)])
                    nc.tensor.matmul(out=ps, lhsT=wx_all[:, j, :], rhs=u_j,
                                     start=(j == 0), stop=(j == DB - 1))
                nc.scalar.copy(out=xdbl_t[:, ts(tf, 512)], in_=ps)
            bc_bf = s2w.tile([32, T], BF16)
            nc.vector.tensor_copy(out=bc_bf, in_=xdbl_t[32:64, :])
            nc.sync.dma_start(out=bc_dram[:], in_=bc_bf)
            # dt = softplus(wdtT.T @ dts + bdt) = -ln(sigmoid(-raw - bdt));
            # store lnS = -dt (bf16). bdt_t holds host-negated bdt.
            for j in range(DB):
                wdt_j = s2.tile([32, 128], F32, tag="wdt")
                nc.sync.dma_start(out=wdt_j, in_=wdtT[:, ts(j, 128)])
                s_sb = s2.tile([128, T], F32, tag="s_sb")
                for tf in range(TF):
                    ps = ps2.tile([128, 512], F32, tag="ps_dt")
                    nc.tensor.matmul(out=ps, lhsT=wdt_j,
                                     rhs=xdbl_t[0:32, ts(tf, 512)],
                                     start=True, stop=True)
                    nc.scalar.activation(out=s_sb[:, ts(tf, 512)], in_=ps,
                                         func=AF.Sigmoid,
                                         bias=bdt_t[:, j:j + 1], scale=-1.0)
                nc.sync.dma_start(out=s_dram[:, j, :], in_=s_sb)
            for j in range(DB):
                s_ld = s2.tile([128, T], F32, tag="s_ld")
                nc.sync.dma_start(out=s_ld, in_=s_dram[:, j, :])
                dt_bf = s2.tile([128, T], BF16, tag="dt_bf")
                nc.scalar.activation(out=dt_bf, in_=s_ld, func=AF.Ln, scale=1.0)
                nc.sync.dma_start(out=dt_dram[:, j, :], in_=dt_bf)

        # ================= Phase 3: selective scan =================
        with tc.tile_pool(name="s3", bufs=2) as s3, \
             tc.tile_pool(name="s3a", bufs=1) as s3a:
            for j in range(DB):
                dt_j = s3.tile([128, T], BF16, tag="dt_j")
                nc.sync.dma_start(out=dt_j, in_=dt_dram[:, j, :])
                u_j = s3.tile([128, T], BF16, tag="u_j")
                nc.sync.dma_start(out=u_j, in_=u_dram[:, j, :])
                sz_j = s3.tile([128, T], BF16, tag="sz_j")
                nc.sync.dma_start(out=sz_j, in_=sz_dram[:, j, :])
                w_j = s3.tile([128, T], BF16, tag="w_j")
                nc.vector.scalar_tensor_tensor(out=w_j, in0=dt_j, scalar=-1.0,
                                               in1=u_j, op0=OP.mult, op1=OP.mult)
                l1 = None
                l2 = None
                l3a = None
                l3b = None
                for k in range(8):  # n pairs
                    br = s3.tile([128, 2, T], BF16, tag="br")
                    cr = s3.tile([128, 2, T], BF16, tag="cr")
                    for i in range(2):
                        nc.sync.dma_start(out=br[:, i, :],
                                          in_=_bcast_row(bc_dram[2 * k + i, :]))
                        nc.sync.dma_start(out=cr[:, i, :],
                                          in_=_bcast_row(bc_dram[N + 2 * k + i, :]))
                    pq = []
                    for i in range(2):
                        n = 2 * k + i
                        dA = s3.tile([128, T], F32, tag="dA")
                        nc.scalar.activation(out=dA, in_=dt_j, func=AF.Exp,
                                             scale=a_t[:, j, n:n + 1])
                        xn = s3.tile([128, T], BF16, tag="xn")
                        xe = nc.vector if n in _N_DVE_XN else nc.gpsimd
                        xe.tensor_tensor(out=xn, in0=w_j, in1=br[:, i, :], op=OP.mult)
                        h = s3.tile([128, T], BF16, tag="h")
                        nc.vector.tensor_tensor_scan(out=h, data0=dA, data1=xn,
                                                     initial=0.0,
                                                     op0=OP.mult, op1=OP.add)
                        p = s3.tile([128, T], BF16, tag=f"p{i}")
                        pe = nc.vector if n in _N_DVE_P else nc.gpsimd
                        pe.tensor_tensor(out=p, in0=h, in1=cr[:, i, :], op=OP.mult)
                        pq.append(p)
                    nl1 = s3a.tile([128, T], BF16, tag=f"l1_{k % 2}")
                    nc.gpsimd.tensor_tensor(out=nl1, in0=pq[0], in1=pq[1], op=OP.add)
                    if k % 2 == 0:
                        l1 = nl1
                    else:
                        nl2 = s3a.tile([128, T], BF16, tag=f"l2_{(k // 2) % 2}")
                        nc.gpsimd.tensor_tensor(out=nl2, in0=l1, in1=nl1, op=OP.add)
                        if k % 4 == 1:
                            l2 = nl2
                        else:
                            nl3 = s3a.tile([128, T], BF16, tag=f"l3_{k // 4}")
                            nc.gpsimd.tensor_tensor(out=nl3, in0=l2, in1=nl2, op=OP.add)
                            if k == 3:
                                l3a = nl3
                            else:
                                l3b = nl3
                ysum = s3a.tile([128, T], F32, tag="ysum")
                nc.vector.tensor_tensor(out=ysum, in0=l3a, in1=l3b, op=OP.add)
                ytot = s3a.tile([128, T], F32, tag="ytot")
                nc.vector.scalar_tensor_tensor(out=ytot, in0=u_j,
                                               scalar=dv_t[:, j:j + 1], in1=ysum,
                                               op0=OP.mult, op1=OP.add)
                nc.vector.tensor_tensor(out=g_t[:, j, :], in0=ytot,
                                        in1=sz_j, op=OP.mult)
                nc.sync.dma_start(out=gsend[j][:], in_=g_t[:, j, :])
                nc.gpsimd.collective_compute(
                    "AllGather", OP.bypass,
                    replica_groups=[[0, 1], [2, 3], [4, 5], [6, 7]],
                    ins=[gsend[j][:].opt()],
                    outs=[ggath[j][:].opt()],
                )

        # ================= Phase 4: P matmul + GLU + GroupNorm =================
        with tc.tile_pool(name="s4", bufs=2) as s4, \
             tc.tile_pool(name="s4g", bufs=1) as s4g, \
             tc.tile_pool(name="ps4", bufs=2, space="PSUM") as ps4:
            goth = s4g.tile([128, DB, T], BF16)
            for j in range(DB):
                nc.sync.dma_start(out=goth[:, j, :], in_=ggath[j][1, :, :])
            yg = s4g.tile([128, CB, T], F32)        # GLU result
            stats = s4g.tile([128, CB * TF, 6], F32)
            for po in range(CB):                    # pair (po, po+4)
                lhs = {}
                for (nm, src, col) in (("oa", mownT, po), ("ta", mothT, po),
                                       ("os", mownT, po + 4), ("tss", mothT, po + 4)):
                    tiles = []
                    for jj in range(DB):
                        lt = s4.tile([128, 128], BF16, tag=f"{nm}{jj}")
                        nc.sync.dma_start(out=lt, in_=src[ts(jj, 128), ts(col, 128)])
                        tiles.append(lt)
                    lhs[nm] = tiles
                for tf in range(TF):
                    ps_a = ps4.tile([128, 512], F32, tag="ps_a")
                    ps_s = ps4.tile([128, 512], F32, tag="ps_s")
                    for jj in range(DB):
                        nc.tensor.matmul(out=ps_a, lhsT=lhs["oa"][jj],
                                         rhs=g_t[:, jj, ts(tf, 512)],
                                         start=(jj == 0), stop=False)
                        nc.tensor.matmul(out=ps_s, lhsT=lhs["os"][jj],
                                         rhs=g_t[:, jj, ts(tf, 512)],
                                         start=(jj == 0), stop=False)
                    for jj in range(DB):
                        nc.tensor.matmul(out=ps_a, lhsT=lhs["ta"][jj],
                                         rhs=goth[:, jj, ts(tf, 512)],
                                         start=False, stop=(jj == DB - 1))
                        nc.tensor.matmul(out=ps_s, lhsT=lhs["tss"][jj],
                                         rhs=goth[:, jj, ts(tf, 512)],
                                         start=False, stop=(jj == DB - 1))
                    ya = s4.tile([128, 512], F32, tag="ya")
                    sig = s4.tile([128, 512], F32, tag="sig")
                    nc.scalar.activation(out=ya, in_=ps_a, func=AF.Identity,
                                         bias=cvb_t[:, po:po + 1], scale=1.0)
                    nc.scalar.activation(out=sig, in_=ps_s, func=AF.Sigmoid,
                                         bias=cvb_t[:, po + 4:po + 5], scale=1.0)
                    nc.vector.tensor_tensor(out=yg[:, po, ts(tf, 512)],
                                            in0=ya, in1=sig, op=OP.mult)
                    nc.vector.bn_stats(out=stats[:, po * TF + tf, :],
                                       in_=yg[:, po, ts(tf, 512)])
            # GroupNorm over all (C, T)
            mv = s4.tile([128, 2], F32, tag="mv")
            nc.vector.bn_aggr(out=mv, in_=stats)
            stat2 = s4.tile([128, 2], F32, tag="stat2")
            nc.vector.tensor_copy(out=stat2[:, 0:1], in_=mv[:, 0:1])
            nc.vector.scalar_tensor_tensor(out=stat2[:, 1:2], in0=mv[:, 0:1],
                                           scalar=mv[:, 0:1], in1=mv[:, 1:2],
                                           op0=OP.mult, op1=OP.add)  # m^2 + v
            ps_st = ps4.tile([2, 1], F32, tag="ps_st")
            nc.tensor.matmul(out=ps_st, lhsT=stat2, rhs=ones_t,
                             start=True, stop=True)
            sc_sb = s4.tile([2, 1], F32, tag="sc_sb")
            nc.scalar.copy(out=sc_sb, in_=ps_st)
            nc.sync.dma_start(out=st_dram[0, :], in_=sc_sb[:, 0:1])
            st_bc = s4.tile([128, 2], F32, tag="st_bc")
            nc.sync.dma_start(out=st_bc, in_=_bcast_row(st_dram[0, :]))
            mu = s4.tile([128, 1], F32, tag="mu")
            e2 = s4.tile([128, 1], F32, tag="e2")
            nc.vector.tensor_scalar(out=mu, in0=st_bc[:, 0:1], scalar1=1.0 / 128,
                                    scalar2=None, op0=OP.mult)
            nc.vector.tensor_scalar(out=e2, in0=st_bc[:, 1:2], scalar1=1.0 / 128,
                                    scalar2=None, op0=OP.mult)
            mu2 = s4.tile([128, 1], F32, tag="mu2")
            nc.vector.tensor_tensor(out=mu2, in0=mu, in1=mu, op=OP.mult)
            var = s4.tile([128, 1], F32, tag="var")
            nc.vector.tensor_tensor(out=var, in0=e2, in1=mu2, op=OP.subtract)
            sdev = s4.tile([128, 1], F32, tag="sdev")
            nc.scalar.activation(out=sdev, in_=var, func=AF.Sqrt,
                                 bias=eps_t, scale=1.0)
            rstd = s4.tile([128, 1], F32, tag="rstd")
            nc.vector.reciprocal(out=rstd, in_=sdev)
            s1c = s4.tile([128, CB], F32, tag="s1c")
            nc.vector.tensor_scalar(out=s1c, in0=gnw_t, scalar1=rstd,
                                    scalar2=None, op0=OP.mult)
            nmu = s4.tile([128, 1], F32, tag="nmu")
            nc.vector.tensor_scalar(out=nmu, in0=mu, scalar1=-1.0,
                                    scalar2=None, op0=OP.mult)
            s2c = s4.tile([128, CB], F32, tag="s2c")
            nc.vector.scalar_tensor_tensor(out=s2c, in0=s1c, scalar=nmu,
                                           in1=gnb_t, op0=OP.mult, op1=OP.add)
            for cb in range(CB):
                o_t = s4.tile([128, T], F32, tag="o_t")
                nc.vector.tensor_scalar(out=o_t, in0=yg[:, cb, :],
                                        scalar1=s1c[:, cb:cb + 1],
                                        scalar2=s2c[:, cb:cb + 1],
                                        op0=OP.mult, op1=OP.add)
                nc.sync.dma_start(out=out[ts(cb, 128), :], in_=o_t)

    nc.finalize()
    return nc


def _prep_core_inputs(inputs):
    """Host-side weight prep -> list of 8 per-core input dicts."""
    f32 = np.float32
    bf16 = ml_dtypes.bfloat16
    x = np.asarray(inputs["x"], f32)
    conv_w = np.asarray(inputs["conv_w"], f32)
    conv_b = np.asarray(inputs["conv_b"], f32)
    gn_w = np.asarray(inputs["gn_w"], f32)
    gn_b = np.asarray(inputs["gn_b"], f32)

    def dirpack(p):
        Win = np.asarray(inputs[p + "Win"], f32)
        convw = np.asarray(inputs[p + "convw"], f32)
        convbv = np.asarray(inputs[p + "convb"], f32)
        Wx = np.asarray(inputs[p + "Wx"], f32)
        Wdt = np.asarray(inputs[p + "Wdt"], f32)
        bdtv = np.asarray(inputs[p + "bdt"], f32)
        Alog = np.asarray(inputs[p + "Alog"], f32)
        Dv = np.asarray(inputs[p + "Dv"], f32)
        Wout = np.asarray(inputs[p + "Wout"], f32)
        negA = np.exp(Alog)   # dA = exp(lnS * negA), lnS = -dt
        return {
            "winT": np.ascontiguousarray(Win.T),
            "wxT": np.ascontiguousarray(Wx.T).astype(bf16),
            "wdtT": np.ascontiguousarray(Wdt.T),
            "a_sc": np.ascontiguousarray(negA.reshape(DB, 128, N).transpose(1, 0, 2)),
            "taps": np.ascontiguousarray(convw[:, 0, :].reshape(DB, 128, 4).transpose(1, 0, 2)),
            "convb": np.ascontiguousarray(convbv.reshape(DB, 128).T),
            "bdt": np.ascontiguousarray(-bdtv.reshape(DB, 128).T),
            "dv": np.ascontiguousarray(Dv.reshape(DB, 128).T),
            "Wout": Wout,
        }

    fw = dirpack("fw_")
    bw = dirpack("bw_")
    Mf = conv_w[:, :C, 0] @ fw.pop("Wout")
    Mb = conv_w[:, C:, 0] @ bw.pop("Wout")
    MfT = np.ascontiguousarray(Mf.T).astype(bf16)
    MbT = np.ascontiguousarray(Mb.T).astype(bf16)

    shared = {
        "cvb": np.ascontiguousarray(conv_b.reshape(DB, 128).T),
        "gnw": np.ascontiguousarray(gn_w.reshape(CB, 128).T),
        "gnb": np.ascontiguousarray(gn_b.reshape(CB, 128).T),
    }
    in_maps = []
    for b in range(4):
        for d in range(2):
            if d == 0:
                m = dict(fw)
                m["x"] = np.ascontiguousarray(x[b])
                m["mownT"] = MfT
                m["mothT"] = MbT
            else:
                m = dict(bw)
                m["x"] = np.ascontiguousarray(x[b][:, ::-1])
                m["mownT"] = MbT
                m["mothT"] = MfT
            m.update(shared)
            in_maps.append(m)
    return in_maps


def kernel(**inputs):
    if "nc" not in _CACHE:
        _CACHE["nc"] = build_program()
    nc = _CACHE["nc"]
    in_maps = _prep_core_inputs(inputs)
    res = run_bass_kernel_spmd(nc, in_maps, list(range(8)))
    out = np.stack([np.asarray(res.results[2 * b]["out"]) for b in range(4)])
    return out.astype(np.float32)


if __name__ == "__main__":
    import reference
    inputs = {k: np.asarray(v) for k, v in reference.setup_inputs().items()}
    got = kernel(**inputs)
    print("out", got.shape, got.dtype)


# revision 12
# speedup vs baseline: 1.0661x; 1.0661x over previous
"""Bidirectional Mamba (BiMambaWrapper) Trainium2 kernel.

Sharding: 8 cores = (batch 0..3) x (direction fw/bw). One uniform SPMD
program; all direction asymmetry is data-driven:
  - bw cores receive host-time-flipped x[b] and the bw weight set.
  - The final 1x1 conv is pre-multiplied into per-direction matrices
    M_f = conv_w[:, :512, 0] @ fw_Wout, M_b = conv_w[:, 512:, 0] @ bw_Wout;
    each core accumulates P = M_own @ g_own + M_oth @ g_oth in PSUM.
  - Each core sends g (no time flip: the reference concatenates the\n    backward branch in its reversed time order) via per-d-block
    AllGather; every core reads gather slot 1, which is the partner for
    even (fw) cores. Odd cores produce garbage epilogues the host drops.
  - Epilogue (bias, GLU, GroupNorm) runs redundantly per pair; the host
    takes even cores' outputs.

Per core: D=1024 d_inner (8 blocks of 128 partitions), T=2048, N=16.
Selective scan via DVE tensor_tensor_scan per (block, n) over [128, 2048].
"""
import sys
sys.path.insert(0, "/opt/trn_rl_repo")

from contextlib import ExitStack

import numpy as np
import ml_dtypes

import concourse.bass as bass
import concourse.mybir as mybir
import concourse.tile as tile
from concourse import bacc
from concourse.bass import ts
from concourse.bass_utils import run_bass_kernel_spmd

F32 = mybir.dt.float32
BF16 = mybir.dt.bfloat16
AF = mybir.ActivationFunctionType
OP = mybir.AluOpType

C = 512          # d_model
T = 2048
D = 1024         # d_inner
N = 16           # d_state
R = 32           # dt_rank
DB = D // 128    # 8 d blocks
CB = C // 128    # 4 c blocks
TF = T // 512    # 4 free chunks for matmuls
EPS = 1e-5

# how many of the 44 movable bf16 TT ops per j go to DVE (rest on Pool)
_N_DVE_XN = {0, 2, 4, 6, 8, 10, 12, 14, 1, 3, 5}   # xn ops on DVE for these n
_N_DVE_P = {0, 2, 4, 6, 8, 10}                      # p ops on DVE for these n

_CACHE = {}


def _rev(ap):
    """Reverse the innermost free dim of a 2-D [partition, free] AP."""
    (pstep, pcount), (fstep, fcount) = ap.ap[0], ap.ap[1]
    return bass.AP(
        tensor=ap.tensor,
        offset=ap.offset + (fcount - 1) * fstep,
        ap=[[pstep, pcount], [-fstep, fcount]],
    )


def _bcast_row(dram_ap, parts=128):
    """Broadcast a 1-D DRAM row AP across `parts` partitions."""
    return bass.AP(tensor=dram_ap.tensor, offset=dram_ap.offset,
                   ap=[[0, parts]] + [list(d) for d in dram_ap.ap])


def build_program():
    nc = bacc.Bacc("TRN2", target_bir_lowering=False)

    x_in = nc.declare_dram_parameter("x", [C, T], F32, isOutput=False)
    winT = nc.declare_dram_parameter("winT", [C, 2 * D], F32, isOutput=False)
    wxT = nc.declare_dram_parameter("wxT", [D, 64], BF16, isOutput=False)
    wdtT = nc.declare_dram_parameter("wdtT", [R, D], F32, isOutput=False)
    mownT = nc.declare_dram_parameter("mownT", [D, D], BF16, isOutput=False)
    mothT = nc.declare_dram_parameter("mothT", [D, D], BF16, isOutput=False)
    a_sc = nc.declare_dram_parameter("a_sc", [128, DB, N], F32, isOutput=False)
    taps = nc.declare_dram_parameter("taps", [128, DB, 4], F32, isOutput=False)
    convb = nc.declare_dram_parameter("convb", [128, DB], F32, isOutput=False)
    bdt = nc.declare_dram_parameter("bdt", [128, DB], F32, isOutput=False)
    dv = nc.declare_dram_parameter("dv", [128, DB], F32, isOutput=False)
    cvb = nc.declare_dram_parameter("cvb", [128, DB], F32, isOutput=False)
    gnw = nc.declare_dram_parameter("gnw", [128, CB], F32, isOutput=False)
    gnb = nc.declare_dram_parameter("gnb", [128, CB], F32, isOutput=False)
    out = nc.declare_dram_parameter("out", [C, T], F32, isOutput=True)

    with tile.TileContext(nc) as tc, ExitStack() as ctx:
        dram = ctx.enter_context(tc.tile_pool(name="dram", bufs=1, space="DRAM"))
        u_dram = dram.tile([128, DB, T], BF16)
        dt_dram = dram.tile([128, DB, T], BF16)
        sz_dram = dram.tile([128, DB, T], BF16)
        s_dram = dram.tile([128, DB, T], F32)
        bc_dram = dram.tile([2 * N, T], BF16)
        st_dram = dram.tile([1, 2], F32)
        gsend = [dram.tile([128, T], BF16, tag=f"gs{j}", name=f"gs{j}") for j in range(DB)]
        ggath = [dram.tile([2, 128, T], BF16, tag=f"gg{j}", name=f"gg{j}")
                 for j in range(DB)]

        consts = ctx.enter_context(tc.tile_pool(name="consts", bufs=1))
        a_t = consts.tile([128, DB, N], F32)
        taps_t = consts.tile([128, DB, 4], F32)
        convb_t = consts.tile([128, DB], F32)
        bdt_t = consts.tile([128, DB], F32)
        dv_t = consts.tile([128, DB], F32)
        cvb_t = consts.tile([128, DB], F32)
        gnw_t = consts.tile([128, CB], F32)
        gnb_t = consts.tile([128, CB], F32)
        ones_t = consts.tile([128, 1], F32)
        eps_t = consts.tile([128, 1], F32)
        nc.vector.memset(eps_t, EPS)
        for t_, s_ in ((a_t, a_sc), (taps_t, taps), (convb_t, convb),
                       (bdt_t, bdt), (dv_t, dv), (cvb_t, cvb),
                       (gnw_t, gnw), (gnb_t, gnb)):
            nc.sync.dma_start(out=t_, in_=s_[:])
        nc.vector.memset(ones_t, 1.0)

        persist = ctx.enter_context(tc.tile_pool(name="persist", bufs=1))
        g_t = persist.tile([128, DB, T], BF16)      # own g (own-time)

        # ================= Phase 1: Win matmul, conv, silu =================
        with tc.tile_pool(name="s1", bufs=2) as s1, \
             tc.tile_pool(name="s1x", bufs=1) as s1x, \
             tc.tile_pool(name="ps1", bufs=2, space="PSUM") as ps1:
            x_sb = s1x.tile([128, CB, T], F32)
            for cb in range(CB):
                nc.sync.dma_start(out=x_sb[:, cb, :], in_=x_in[ts(cb, 128), :])
            for j in range(DB):
                # xc block j: lhsT = winT[:, j*128:(j+1)*128]
                lws = []
                for cb in range(CB):
                    lw = s1.tile([128, 128], F32, tag=f"lw{cb}")
                    nc.sync.dma_start(out=lw, in_=winT[ts(cb, 128), ts(j, 128)])
                    lws.append(lw)
                xc = s1.tile([128, 3 + T], F32, tag="xc")
                nc.vector.memset(xc[:, 0:3], 0.0)
                for tf in range(TF):
                    ps = ps1.tile([128, 512], F32, tag="ps_xc")
                    for cb in range(CB):
                        nc.tensor.matmul(out=ps, lhsT=lws[cb],
                                         rhs=x_sb[:, cb, ts(tf, 512)],
                                         start=(cb == 0), stop=(cb == CB - 1))
                    nc.scalar.copy(out=xc[:, 3 + tf * 512: 3 + (tf + 1) * 512], in_=ps)
                # depthwise causal conv: out[t] = sum_k taps[k]*xc[t-3+k] + convb
                cv = s1.tile([128, T], F32, tag="cv")
                t0 = s1.tile([128, T], F32, tag="t0")
                nc.vector.tensor_scalar(out=t0, in0=xc[:, 0:T],
                                        scalar1=taps_t[:, j, 0:1], scalar2=None,
                                        op0=OP.mult)
                nc.vector.scalar_tensor_tensor(out=cv, in0=xc[:, 1:1 + T],
                                               scalar=taps_t[:, j, 1:2], in1=t0,
                                               op0=OP.mult, op1=OP.add)
                nc.vector.scalar_tensor_tensor(out=t0, in0=xc[:, 2:2 + T],
                                               scalar=taps_t[:, j, 2:3], in1=cv,
                                               op0=OP.mult, op1=OP.add)
                nc.vector.scalar_tensor_tensor(out=cv, in0=xc[:, 3:3 + T],
                                               scalar=taps_t[:, j, 3:4], in1=t0,
                                               op0=OP.mult, op1=OP.add)
                u_bf = s1.tile([128, T], BF16, tag="u_bf")
                nc.scalar.activation(out=u_bf, in_=cv, func=AF.Silu,
                                     bias=convb_t[:, j:j + 1], scale=1.0)
                nc.sync.dma_start(out=u_dram[:, j, :], in_=u_bf)
                # z block j: Win rows D + j*128 -> silu -> spill
                lwzs = []
                for cb in range(CB):
                    lwz = s1.tile([128, 128], F32, tag=f"lwz{cb}")
                    nc.sync.dma_start(out=lwz,
                                      in_=winT[ts(cb, 128), D + j * 128: D + (j + 1) * 128])
                    lwzs.append(lwz)
                for tf in range(TF):
                    ps = ps1.tile([128, 512], F32, tag="ps_z")
                    for cb in range(CB):
                        nc.tensor.matmul(out=ps, lhsT=lwzs[cb],
                                         rhs=x_sb[:, cb, ts(tf, 512)],
                                         start=(cb == 0), stop=(cb == CB - 1))
                    szt = s1.tile([128, 512], BF16, tag="szt")
                    nc.scalar.activation(out=szt, in_=ps, func=AF.Silu, scale=1.0)
                    nc.sync.dma_start(out=sz_dram[:, j, ts(tf, 512)], in_=szt)

        # ================= Phase 2: x_dbl, dt, B/C bounce =================
        with tc.tile_pool(name="s2", bufs=2) as s2, \
             tc.tile_pool(name="s2w", bufs=1) as s2w, \
             tc.tile_pool(name="ps2", bufs=2, space="PSUM") as ps2:
            wx_all = s2w.tile([128, DB, 64], BF16)
            for j in range(DB):
                nc.sync.dma_start(out=wx_all[:, j, :], in_=wxT[ts(j, 128), :])
            xdbl_t = s2w.tile([64, T], F32)
            for tf in range(TF):
                ps = ps2.tile([64, 512], F32, tag="ps_xd")
                for j in range(DB):
                    u_j = s2.tile([128, 512], BF16, tag="u_s")
                    nc.sync.dma_start(out=u_j, in_=u_dram[:, j, ts(tf, 512)])
                    nc.tensor.matmul(out=ps, lhsT=wx_all[:, j, :], rhs=u_j,
                                     start=(j == 0), stop=(j == DB - 1))
                nc.scalar.copy(out=xdbl_t[:, ts(tf, 512)], in_=ps)
            bc_bf = s2w.tile([32, T], BF16)
            nc.vector.tensor_copy(out=bc_bf, in_=xdbl_t[32:64, :])
            nc.sync.dma_start(out=bc_dram[:], in_=bc_bf)
            # dt = softplus(wdtT.T @ dts + bdt) = -ln(sigmoid(-raw - bdt));
            # store lnS = -dt (bf16). bdt_t holds host-negated bdt.
            for j in range(DB):
                wdt_j = s2.tile([32, 128], F32, tag="wdt")
                nc.sync.dma_start(out=wdt_j, in_=wdtT[:, ts(j, 128)])
                s_sb = s2.tile([128, T], F32, tag="s_sb")
                for tf in range(TF):
                    ps = ps2.tile([128, 512], F32, tag="ps_dt")
                    nc.tensor.matmul(out=ps, lhsT=wdt_j,
                                     rhs=xdbl_t[0:32, ts(tf, 512)],
                                     start=True, stop=True)
                    nc.scalar.activation(out=s_sb[:, ts(tf, 512)], in_=ps,
                                         func=AF.Sigmoid,
                                         bias=bdt_t[:, j:j + 1], scale=-1.0)
                nc.sync.dma_start(out=s_dram[:, j, :], in_=s_sb)
            for j in range(DB):
                s_ld = s2.tile([128, T], F32, tag="s_ld")
                nc.sync.dma_start(out=s_ld, in_=s_dram[:, j, :])
                dt_bf = s2.tile([128, T], BF16, tag="dt_bf")
                nc.scalar.activation(out=dt_bf, in_=s_ld, func=AF.Ln, scale=1.0)
                nc.sync.dma_start(out=dt_dram[:, j, :], in_=dt_bf)

        # ================= Phase 3: selective scan =================
        with tc.tile_pool(name="s3", bufs=2) as s3, \
             tc.tile_pool(name="s3a", bufs=1) as s3a:
            for j in range(DB):
                dt_j = s3.tile([128, T], BF16, tag="dt_j")
                nc.sync.dma_start(out=dt_j, in_=dt_dram[:, j, :])
                u_j = s3.tile([128, T], BF16, tag="u_j")
                nc.sync.dma_start(out=u_j, in_=u_dram[:, j, :])
                sz_j = s3.tile([128, T], BF16, tag="sz_j")
                nc.sync.dma_start(out=sz_j, in_=sz_dram[:, j, :])
                w_j = s3.tile([128, T], BF16, tag="w_j")
                nc.vector.scalar_tensor_tensor(out=w_j, in0=dt_j, scalar=-1.0,
                                               in1=u_j, op0=OP.mult, op1=OP.mult)
                l1 = None
                l2 = None
                l3a = None
                l3b = None
                for k in range(8):  # n pairs
                    br = s3.tile([128, 2, T], BF16, tag="br", bufs=1)
                    cr = s3.tile([128, 2, T], BF16, tag="cr", bufs=1)
                    nc.sync.dma_start(out=br, in_=_bcast_row(bc_dram[2 * k:2 * k + 2, :]))
                    nc.sync.dma_start(out=cr, in_=_bcast_row(bc_dram[N + 2 * k:N + 2 * k + 2, :]))
                    dA2 = s3.tile([128, 2, T], F32, tag="dA2")
                    xn2 = s3.tile([128, 2, T], BF16, tag="xn2")
                    for i in range(2):
                        n = 2 * k + i
                        nc.scalar.activation(out=dA2[:, i, :], in_=dt_j, func=AF.Exp,
                                             scale=a_t[:, j, n:n + 1])
                        if i == 1:
                            # series-2 start: dA=0 resets the packed-scan state
                            nc.vector.memset(dA2[:, 1, 0:1], 0.0)
                        nc.vector.tensor_tensor(out=xn2[:, i, :], in0=w_j,
                                                in1=br[:, i, :], op=OP.mult)
                    h2 = s3.tile([128, 2, T], BF16, tag="h2")
                    nc.vector.tensor_tensor_scan(
                        out=h2.rearrange("p a b -> p (a b)"),
                        data0=dA2.rearrange("p a b -> p (a b)"),
                        data1=xn2.rearrange("p a b -> p (a b)"),
                        initial=0.0, op0=OP.mult, op1=OP.add)
                    p2 = s3.tile([128, 2, T], BF16, tag="p2")
                    pe = nc.vector if k % 2 == 0 else nc.gpsimd
                    pe.tensor_tensor(out=p2.rearrange("p a b -> p (a b)"),
                                     in0=h2.rearrange("p a b -> p (a b)"),
                                     in1=cr.rearrange("p a b -> p (a b)"), op=OP.mult)
                    nl1 = s3a.tile([128, T], BF16, tag=f"l1_{k % 2}")
                    nc.gpsimd.tensor_tensor(out=nl1, in0=p2[:, 0, :], in1=p2[:, 1, :],
                                            op=OP.add)
                    if k % 2 == 0:
                        l1 = nl1
                    else:
                        nl2 = s3a.tile([128, T], BF16, tag=f"l2_{(k // 2) % 2}")
                        nc.gpsimd.tensor_tensor(out=nl2, in0=l1, in1=nl1, op=OP.add)
                        if k % 4 == 1:
                            l2 = nl2
                        else:
                            nl3 = s3a.tile([128, T], BF16, tag=f"l3_{k // 4}")
                            nc.gpsimd.tensor_tensor(out=nl3, in0=l2, in1=nl2, op=OP.add)
                            if k == 3:
                                l3a = nl3
                            else:
                                l3b = nl3
                ysum = s3a.tile([128, T], BF16, tag="ysum")
                nc.vector.tensor_tensor(out=ysum, in0=l3a, in1=l3b, op=OP.add)
                ytot = s3a.tile([128, T], BF16, tag="ytot")
                nc.vector.scalar_tensor_tensor(out=ytot, in0=u_j,
                                               scalar=dv_t[:, j:j + 1], in1=ysum,
                                               op0=OP.mult, op1=OP.add)
                nc.vector.tensor_tensor(out=g_t[:, j, :], in0=ytot,
                                        in1=sz_j, op=OP.mult)
                nc.sync.dma_start(out=gsend[j][:], in_=g_t[:, j, :])
                nc.gpsimd.collective_compute(
                    "AllGather", OP.bypass,
                    replica_groups=[[0, 1], [2, 3], [4, 5], [6, 7]],
                    ins=[gsend[j][:].opt()],
                    outs=[ggath[j][:].opt()],
                )

        # ================= Phase 4: P matmul + GLU + GroupNorm =================
        with tc.tile_pool(name="s4", bufs=2) as s4, \
             tc.tile_pool(name="s4g", bufs=1) as s4g, \
             tc.tile_pool(name="ps4", bufs=2, space="PSUM") as ps4:
            goth = s4g.tile([128, DB, T], BF16)
            for j in range(DB):
                nc.sync.dma_start(out=goth[:, j, :], in_=ggath[j][1, :, :])
            yg = s4g.tile([128, CB, T], F32)        # GLU result
            stats = s4g.tile([128, CB * TF, 6], F32)
            for po in range(CB):                    # pair (po, po+4)
                lhs = {}
                for (nm, src, col) in (("oa", mownT, po), ("ta", mothT, po),
                                       ("os", mownT, po + 4), ("tss", mothT, po + 4)):
                    tiles = []
                    for jj in range(DB):
                        lt = s4.tile([128, 128], BF16, tag=f"{nm}{jj}")
                        nc.sync.dma_start(out=lt, in_=src[ts(jj, 128), ts(col, 128)])
                        tiles.append(lt)
                    lhs[nm] = tiles
                for tf in range(TF):
                    ps_a = ps4.tile([128, 512], F32, tag="ps_a")
                    ps_s = ps4.tile([128, 512], F32, tag="ps_s")
                    for jj in range(DB):
                        nc.tensor.matmul(out=ps_a, lhsT=lhs["oa"][jj],
                                         rhs=g_t[:, jj, ts(tf, 512)],
                                         start=(jj == 0), stop=False)
                        nc.tensor.matmul(out=ps_s, lhsT=lhs["os"][jj],
                                         rhs=g_t[:, jj, ts(tf, 512)],
                                         start=(jj == 0), stop=False)
                    for jj in range(DB):
                        nc.tensor.matmul(out=ps_a, lhsT=lhs["ta"][jj],
                                         rhs=goth[:, jj, ts(tf, 512)],
                                         start=False, stop=(jj == DB - 1))
                        nc.tensor.matmul(out=ps_s, lhsT=lhs["tss"][jj],
                                         rhs=goth[:, jj, ts(tf, 512)],
                                         start=False, stop=(jj == DB - 1))
                    ya = s4.tile([128, 512], F32, tag="ya")
                    sig = s4.tile([128, 512], F32, tag="sig")
                    nc.scalar.activation(out=ya, in_=ps_a, func=AF.Identity,
                                         bias=cvb_t[:, po:po + 1], scale=1.0)
                    nc.scalar.activation(out=sig, in_=ps_s, func=AF.Sigmoid,
                                         bias=cvb_t[:, po + 4:po + 5], scale=1.0)
                    nc.vector.tensor_tensor(out=yg[:, po, ts(tf, 512)],
                                            in0=ya, in1=sig, op=OP.mult)
                    nc.vector.bn_stats(out=stats[:, po * TF + tf, :],
                                       in_=yg[:, po, ts(tf, 512)])
            # GroupNorm over all (C, T)
            mv = s4.tile([128, 2], F32, tag="mv")
            nc.vector.bn_aggr(out=mv, in_=stats)
            stat2 = s4.tile([128, 2], F32, tag="stat2")
            nc.vector.tensor_copy(out=stat2[:, 0:1], in_=mv[:, 0:1])
            nc.vector.scalar_tensor_tensor(out=stat2[:, 1:2], in0=mv[:, 0:1],
                                           scalar=mv[:, 0:1], in1=mv[:, 1:2],
                                           op0=OP.mult, op1=OP.add)  # m^2 + v
            ps_st = ps4.tile([2, 1], F32, tag="ps_st")
            nc.tensor.matmul(out=ps_st, lhsT=stat2, rhs=ones_t,
                             start=True, stop=True)
            sc_sb = s4.tile([2, 1], F32, tag="sc_sb")
            nc.scalar.copy(out=sc_sb, in_=ps_st)
            nc.sync.dma_start(out=st_dram[0, :], in_=sc_sb[:, 0:1])
            st_bc = s4.tile([128, 2], F32, tag="st_bc")
            nc.sync.dma_start(out=st_bc, in_=_bcast_row(st_dram[0, :]))
            mu = s4.tile([128, 1], F32, tag="mu")
            e2 = s4.tile([128, 1], F32, tag="e2")
            nc.vector.tensor_scalar(out=mu, in0=st_bc[:, 0:1], scalar1=1.0 / 128,
                                    scalar2=None, op0=OP.mult)
            nc.vector.tensor_scalar(out=e2, in0=st_bc[:, 1:2], scalar1=1.0 / 128,
                                    scalar2=None, op0=OP.mult)
            mu2 = s4.tile([128, 1], F32, tag="mu2")
            nc.vector.tensor_tensor(out=mu2, in0=mu, in1=mu, op=OP.mult)
            var = s4.tile([128, 1], F32, tag="var")
            nc.vector.tensor_tensor(out=var, in0=e2, in1=mu2, op=OP.subtract)
            sdev = s4.tile([128, 1], F32, tag="sdev")
            nc.scalar.activation(out=sdev, in_=var, func=AF.Sqrt,
                                 bias=eps_t, scale=1.0)
            rstd = s4.tile([128, 1], F32, tag="rstd")
            nc.vector.reciprocal(out=rstd, in_=sdev)
            s1c = s4.tile([128, CB], F32, tag="s1c")
            nc.vector.tensor_scalar(out=s1c, in0=gnw_t, scalar1=rstd,
                                    scalar2=None, op0=OP.mult)
            nmu = s4.tile([128, 1], F32, tag="nmu")
            nc.vector.tensor_scalar(out=nmu, in0=mu, scalar1=-1.0,
                                    scalar2=None, op0=OP.mult)
            s2c = s4.tile([128, CB], F32, tag="s2c")
            nc.vector.scalar_tensor_tensor(out=s2c, in0=s1c, scalar=nmu,
                                           in1=gnb_t, op0=OP.mult, op1=OP.add)
            for cb in range(CB):
                o_t = s4.tile([128, T], F32, tag="o_t")
                nc.vector.tensor_scalar(out=o_t, in0=yg[:, cb, :],
                                        scalar1=s1c[:, cb:cb + 1],
                                        scalar2=s2c[:, cb:cb + 1],
                                        op0=OP.mult, op1=OP.add)
                nc.sync.dma_start(out=out[ts(cb, 128), :], in_=o_t)

    nc.finalize()
    return nc


def _prep_core_inputs(inputs):
    """Host-side weight prep -> list of 8 per-core input dicts."""
    f32 = np.float32
    bf16 = ml_dtypes.bfloat16
    x = np.asarray(inputs["x"], f32)
    conv_w = np.asarray(inputs["conv_w"], f32)
    conv_b = np.asarray(inputs["conv_b"], f32)
    gn_w = np.asarray(inputs["gn_w"], f32)
    gn_b = np.asarray(inputs["gn_b"], f32)

    def dirpack(p):
        Win = np.asarray(inputs[p + "Win"], f32)
        convw = np.asarray(inputs[p + "convw"], f32)
        convbv = np.asarray(inputs[p + "convb"], f32)
        Wx = np.asarray(inputs[p + "Wx"], f32)
        Wdt = np.asarray(inputs[p + "Wdt"], f32)
        bdtv = np.asarray(inputs[p + "bdt"], f32)
        Alog = np.asarray(inputs[p + "Alog"], f32)
        Dv = np.asarray(inputs[p + "Dv"], f32)
        Wout = np.asarray(inputs[p + "Wout"], f32)
        negA = np.exp(Alog)   # dA = exp(lnS * negA), lnS = -dt
        return {
            "winT": np.ascontiguousarray(Win.T),
            "wxT": np.ascontiguousarray(Wx.T).astype(bf16),
            "wdtT": np.ascontiguousarray(Wdt.T),
            "a_sc": np.ascontiguousarray(negA.reshape(DB, 128, N).transpose(1, 0, 2)),
            "taps": np.ascontiguousarray(convw[:, 0, :].reshape(DB, 128, 4).transpose(1, 0, 2)),
            "convb": np.ascontiguousarray(convbv.reshape(DB, 128).T),
            "bdt": np.ascontiguousarray(-bdtv.reshape(DB, 128).T),
            "dv": np.ascontiguousarray(Dv.reshape(DB, 128).T),
            "Wout": Wout,
        }

    fw = dirpack("fw_")
    bw = dirpack("bw_")
    Mf = conv_w[:, :C, 0] @ fw.pop("Wout")
    Mb = conv_w[:, C:, 0] @ bw.pop("Wout")
    MfT = np.ascontiguousarray(Mf.T).astype(bf16)
    MbT = np.ascontiguousarray(Mb.T).astype(bf16)

    shared = {
        "cvb": np.ascontiguousarray(conv_b.reshape(DB, 128).T),
        "gnw": np.ascontiguousarray(gn_w.reshape(CB, 128).T),
        "gnb": np.ascontiguousarray(gn_b.reshape(CB, 128).T),
    }
    in_maps = []
    for b in range(4):
        for d in range(2):
            if d == 0:
                m = dict(fw)
                m["x"] = np.ascontiguousarray(x[b])
                m["mownT"] = MfT
                m["mothT"] = MbT
            else:
                m = dict(bw)
                m["x"] = np.ascontiguousarray(x[b][:, ::-1])
                m["mownT"] = MbT
                m["mothT"] = MfT
            m.update(shared)
            in_maps.append(m)
    return in_maps


def kernel(**inputs):
    if "nc" not in _CACHE:
        _CACHE["nc"] = build_program()
    nc = _CACHE["nc"]
    in_maps = _prep_core_inputs(inputs)
    res = run_bass_kernel_spmd(nc, in_maps, list(range(8)))
    out = np.stack([np.asarray(res.results[2 * b]["out"]) for b in range(4)])
    return out.astype(np.float32)


if __name__ == "__main__":
    import reference
    inputs = {k: np.asarray(v) for k, v in reference.setup_inputs().items()}
    got = kernel(**inputs)
    print("out", got.shape, got.dtype)


# revision 13
# speedup vs baseline: 1.1573x; 1.0855x over previous
"""Bidirectional Mamba (BiMambaWrapper) Trainium2 kernel.

Sharding: 8 cores = (batch 0..3) x (direction fw/bw). One uniform SPMD
program; all direction asymmetry is data-driven:
  - bw cores receive host-time-flipped x[b] and the bw weight set.
  - The final 1x1 conv is pre-multiplied into per-direction matrices
    M_f = conv_w[:, :512, 0] @ fw_Wout, M_b = conv_w[:, 512:, 0] @ bw_Wout;
    each core accumulates P = M_own @ g_own + M_oth @ g_oth in PSUM.
  - Each core sends g (no time flip: the reference concatenates the\n    backward branch in its reversed time order) via per-d-block
    AllGather; every core reads gather slot 1, which is the partner for
    even (fw) cores. Odd cores produce garbage epilogues the host drops.
  - Epilogue (bias, GLU, GroupNorm) runs redundantly per pair; the host
    takes even cores' outputs.

Per core: D=1024 d_inner (8 blocks of 128 partitions), T=2048, N=16.
Selective scan via DVE tensor_tensor_scan per (block, n) over [128, 2048].
"""
import sys
sys.path.insert(0, "/opt/trn_rl_repo")

from contextlib import ExitStack

import numpy as np
import ml_dtypes

import concourse.bass as bass
import concourse.mybir as mybir
import concourse.tile as tile
from concourse import bacc
from concourse.bass import ts
from concourse.bass_utils import run_bass_kernel_spmd

F32 = mybir.dt.float32
BF16 = mybir.dt.bfloat16
AF = mybir.ActivationFunctionType
OP = mybir.AluOpType

C = 512          # d_model
T = 2048
D = 1024         # d_inner
N = 16           # d_state
R = 32           # dt_rank
DB = D // 128    # 8 d blocks
CB = C // 128    # 4 c blocks
TF = T // 512    # 4 free chunks for matmuls
EPS = 1e-5

# how many of the 44 movable bf16 TT ops per j go to DVE (rest on Pool)
_N_DVE_XN = {0, 2, 4, 6, 8, 10, 12, 14, 1, 3, 5}   # xn ops on DVE for these n
_N_DVE_P = {0, 2, 4, 6, 8, 10}                      # p ops on DVE for these n

_CACHE = {}


def _rev(ap):
    """Reverse the innermost free dim of a 2-D [partition, free] AP."""
    (pstep, pcount), (fstep, fcount) = ap.ap[0], ap.ap[1]
    return bass.AP(
        tensor=ap.tensor,
        offset=ap.offset + (fcount - 1) * fstep,
        ap=[[pstep, pcount], [-fstep, fcount]],
    )


def _bcast_row(dram_ap, parts=128):
    """Broadcast a 1-D DRAM row AP across `parts` partitions."""
    return bass.AP(tensor=dram_ap.tensor, offset=dram_ap.offset,
                   ap=[[0, parts]] + [list(d) for d in dram_ap.ap])


def build_program():
    nc = bacc.Bacc("TRN2", target_bir_lowering=False)

    x_in = nc.declare_dram_parameter("x", [C, T], F32, isOutput=False)
    winT = nc.declare_dram_parameter("winT", [C, 2 * D], F32, isOutput=False)
    wxT = nc.declare_dram_parameter("wxT", [D, 64], BF16, isOutput=False)
    wdtT = nc.declare_dram_parameter("wdtT", [R, D], F32, isOutput=False)
    mownT = nc.declare_dram_parameter("mownT", [D, D], BF16, isOutput=False)
    mothT = nc.declare_dram_parameter("mothT", [D, D], BF16, isOutput=False)
    a_sc = nc.declare_dram_parameter("a_sc", [128, DB, N], F32, isOutput=False)
    taps = nc.declare_dram_parameter("taps", [128, DB, 4], F32, isOutput=False)
    convb = nc.declare_dram_parameter("convb", [128, DB], F32, isOutput=False)
    bdt = nc.declare_dram_parameter("bdt", [128, DB], F32, isOutput=False)
    dv = nc.declare_dram_parameter("dv", [128, DB], F32, isOutput=False)
    cvb = nc.declare_dram_parameter("cvb", [128, DB], F32, isOutput=False)
    gnw = nc.declare_dram_parameter("gnw", [128, CB], F32, isOutput=False)
    gnb = nc.declare_dram_parameter("gnb", [128, CB], F32, isOutput=False)
    out = nc.declare_dram_parameter("out", [C, T], F32, isOutput=True)

    with tile.TileContext(nc) as tc, ExitStack() as ctx:
        dram = ctx.enter_context(tc.tile_pool(name="dram", bufs=1, space="DRAM"))
        u_dram = dram.tile([128, DB, T], BF16)
        dt_dram = dram.tile([128, DB, T], BF16)
        sz_dram = dram.tile([128, DB, T], BF16)
        s_dram = dram.tile([128, DB, T], F32)
        bc_dram = dram.tile([2 * N, T], BF16)
        st_dram = dram.tile([1, 2], F32)
        gsend = [dram.tile([128, T], BF16, tag=f"gs{j}", name=f"gs{j}") for j in range(DB)]
        ggath = [dram.tile([2, 128, T], BF16, tag=f"gg{j}", name=f"gg{j}")
                 for j in range(DB)]

        consts = ctx.enter_context(tc.tile_pool(name="consts", bufs=1))
        a_t = consts.tile([128, DB, N], F32)
        taps_t = consts.tile([128, DB, 4], F32)
        convb_t = consts.tile([128, DB], F32)
        bdt_t = consts.tile([128, DB], F32)
        dv_t = consts.tile([128, DB], F32)
        cvb_t = consts.tile([128, DB], F32)
        gnw_t = consts.tile([128, CB], F32)
        gnb_t = consts.tile([128, CB], F32)
        ones_t = consts.tile([128, 1], F32)
        eps_t = consts.tile([128, 1], F32)
        nc.vector.memset(eps_t, EPS)
        for t_, s_ in ((a_t, a_sc), (taps_t, taps), (convb_t, convb),
                       (bdt_t, bdt), (dv_t, dv), (cvb_t, cvb),
                       (gnw_t, gnw), (gnb_t, gnb)):
            nc.sync.dma_start(out=t_, in_=s_[:])
        nc.vector.memset(ones_t, 1.0)

        persist = ctx.enter_context(tc.tile_pool(name="persist", bufs=1))
        g_t = persist.tile([128, DB, T], BF16)      # own g (own-time)

        # ================= Phase 1: Win matmul, conv, silu =================
        with tc.tile_pool(name="s1", bufs=2) as s1, \
             tc.tile_pool(name="s1x", bufs=1) as s1x, \
             tc.tile_pool(name="ps1", bufs=2, space="PSUM") as ps1:
            x_sb = s1x.tile([128, CB, T], F32)
            for cb in range(CB):
                nc.sync.dma_start(out=x_sb[:, cb, :], in_=x_in[ts(cb, 128), :])
            for j in range(DB):
                # xc block j: lhsT = winT[:, j*128:(j+1)*128]
                lws = []
                for cb in range(CB):
                    lw = s1.tile([128, 128], F32, tag=f"lw{cb}")
                    nc.sync.dma_start(out=lw, in_=winT[ts(cb, 128), ts(j, 128)])
                    lws.append(lw)
                xc = s1.tile([128, 3 + T], F32, tag="xc")
                nc.vector.memset(xc[:, 0:3], 0.0)
                for tf in range(TF):
                    ps = ps1.tile([128, 512], F32, tag="ps_xc")
                    for cb in range(CB):
                        nc.tensor.matmul(out=ps, lhsT=lws[cb],
                                         rhs=x_sb[:, cb, ts(tf, 512)],
                                         start=(cb == 0), stop=(cb == CB - 1))
                    nc.scalar.copy(out=xc[:, 3 + tf * 512: 3 + (tf + 1) * 512], in_=ps)
                # depthwise causal conv: out[t] = sum_k taps[k]*xc[t-3+k] + convb
                cv = s1.tile([128, T], F32, tag="cv")
                t0 = s1.tile([128, T], F32, tag="t0")
                nc.vector.tensor_scalar(out=t0, in0=xc[:, 0:T],
                                        scalar1=taps_t[:, j, 0:1], scalar2=None,
                                        op0=OP.mult)
                nc.vector.scalar_tensor_tensor(out=cv, in0=xc[:, 1:1 + T],
                                               scalar=taps_t[:, j, 1:2], in1=t0,
                                               op0=OP.mult, op1=OP.add)
                nc.vector.scalar_tensor_tensor(out=t0, in0=xc[:, 2:2 + T],
                                               scalar=taps_t[:, j, 2:3], in1=cv,
                                               op0=OP.mult, op1=OP.add)
                nc.vector.scalar_tensor_tensor(out=cv, in0=xc[:, 3:3 + T],
                                               scalar=taps_t[:, j, 3:4], in1=t0,
                                               op0=OP.mult, op1=OP.add)
                u_bf = s1.tile([128, T], BF16, tag="u_bf")
                nc.scalar.activation(out=u_bf, in_=cv, func=AF.Silu,
                                     bias=convb_t[:, j:j + 1], scale=1.0)
                nc.sync.dma_start(out=u_dram[:, j, :], in_=u_bf)
                # z block j: Win rows D + j*128 -> silu -> spill
                lwzs = []
                for cb in range(CB):
                    lwz = s1.tile([128, 128], F32, tag=f"lwz{cb}")
                    nc.sync.dma_start(out=lwz,
                                      in_=winT[ts(cb, 128), D + j * 128: D + (j + 1) * 128])
                    lwzs.append(lwz)
                for tf in range(TF):
                    ps = ps1.tile([128, 512], F32, tag="ps_z")
                    for cb in range(CB):
                        nc.tensor.matmul(out=ps, lhsT=lwzs[cb],
                                         rhs=x_sb[:, cb, ts(tf, 512)],
                                         start=(cb == 0), stop=(cb == CB - 1))
                    szt = s1.tile([128, 512], BF16, tag="szt")
                    nc.scalar.activation(out=szt, in_=ps, func=AF.Silu, scale=1.0)
                    nc.sync.dma_start(out=sz_dram[:, j, ts(tf, 512)], in_=szt)

        # ================= Phase 2: x_dbl, dt, B/C bounce =================
        with tc.tile_pool(name="s2", bufs=2) as s2, \
             tc.tile_pool(name="s2w", bufs=1) as s2w, \
             tc.tile_pool(name="ps2", bufs=2, space="PSUM") as ps2:
            wx_all = s2w.tile([128, DB, 64], BF16)
            for j in range(DB):
                nc.sync.dma_start(out=wx_all[:, j, :], in_=wxT[ts(j, 128), :])
            xdbl_t = s2w.tile([64, T], F32)
            for tf in range(TF):
                ps = ps2.tile([64, 512], F32, tag="ps_xd")
                for j in range(DB):
                    u_j = s2.tile([128, 512], BF16, tag="u_s")
                    nc.sync.dma_start(out=u_j, in_=u_dram[:, j, ts(tf, 512)])
                    nc.tensor.matmul(out=ps, lhsT=wx_all[:, j, :], rhs=u_j,
                                     start=(j == 0), stop=(j == DB - 1))
                nc.scalar.copy(out=xdbl_t[:, ts(tf, 512)], in_=ps)
            bc_bf = s2w.tile([32, T], BF16)
            nc.vector.tensor_copy(out=bc_bf, in_=xdbl_t[32:64, :])
            nc.sync.dma_start(out=bc_dram[:], in_=bc_bf)
            # dt = softplus(wdtT.T @ dts + bdt) = -ln(sigmoid(-raw - bdt));
            # store lnS = -dt (bf16). bdt_t holds host-negated bdt.
            for j in range(DB):
                wdt_j = s2.tile([32, 128], F32, tag="wdt")
                nc.sync.dma_start(out=wdt_j, in_=wdtT[:, ts(j, 128)])
                s_sb = s2.tile([128, T], F32, tag="s_sb")
                for tf in range(TF):
                    ps = ps2.tile([128, 512], F32, tag="ps_dt")
                    nc.tensor.matmul(out=ps, lhsT=wdt_j,
                                     rhs=xdbl_t[0:32, ts(tf, 512)],
                                     start=True, stop=True)
                    nc.scalar.activation(out=s_sb[:, ts(tf, 512)], in_=ps,
                                         func=AF.Sigmoid,
                                         bias=bdt_t[:, j:j + 1], scale=-1.0)
                nc.sync.dma_start(out=s_dram[:, j, :], in_=s_sb)
            for j in range(DB):
                s_ld = s2.tile([128, T], F32, tag="s_ld")
                nc.sync.dma_start(out=s_ld, in_=s_dram[:, j, :])
                dt_bf = s2.tile([128, T], BF16, tag="dt_bf")
                nc.scalar.activation(out=dt_bf, in_=s_ld, func=AF.Ln, scale=1.0)
                nc.sync.dma_start(out=dt_dram[:, j, :], in_=dt_bf)

        # ================= Phase 3: selective scan =================
        with tc.tile_pool(name="s3", bufs=2) as s3, \
             tc.tile_pool(name="s3a", bufs=1) as s3a:
            for j in range(DB):
                dt_j = s3.tile([128, T], BF16, tag="dt_j")
                nc.sync.dma_start(out=dt_j, in_=dt_dram[:, j, :])
                u_j = s3.tile([128, T], BF16, tag="u_j")
                nc.sync.dma_start(out=u_j, in_=u_dram[:, j, :])
                sz_j = s3.tile([128, T], BF16, tag="sz_j")
                nc.sync.dma_start(out=sz_j, in_=sz_dram[:, j, :])
                w_j = s3.tile([128, T], BF16, tag="w_j")
                nc.vector.scalar_tensor_tensor(out=w_j, in0=dt_j, scalar=-1.0,
                                               in1=u_j, op0=OP.mult, op1=OP.mult)
                l1 = None
                l2 = None
                l3a = None
                l3b = None
                for k in range(8):  # n pairs
                    br = s3.tile([128, 2, T], BF16, tag="br", bufs=1)
                    cr = s3.tile([128, 2, T], BF16, tag="cr", bufs=1)
                    nc.sync.dma_start(out=br, in_=_bcast_row(bc_dram[2 * k:2 * k + 2, :]))
                    nc.sync.dma_start(out=cr, in_=_bcast_row(bc_dram[N + 2 * k:N + 2 * k + 2, :]))
                    dA2 = s3.tile([128, 2, T], F32, tag="dA2")
                    xn2 = s3.tile([128, 2, T], BF16, tag="xn2", bufs=3)
                    for i in range(2):
                        n = 2 * k + i
                        nc.scalar.activation(out=dA2[:, i, :], in_=dt_j, func=AF.Exp,
                                             scale=a_t[:, j, n:n + 1])
                        if i == 1:
                            # series-2 start: dA=0 resets the packed-scan state
                            nc.vector.memset(dA2[:, 1, 0:1], 0.0)
                        nc.gpsimd.tensor_tensor(out=xn2[:, i, :], in0=w_j,
                                                in1=br[:, i, :], op=OP.mult)
                    h2 = s3.tile([128, 2, T], BF16, tag="h2")
                    nc.vector.tensor_tensor_scan(
                        out=h2.rearrange("p a b -> p (a b)"),
                        data0=dA2.rearrange("p a b -> p (a b)"),
                        data1=xn2.rearrange("p a b -> p (a b)"),
                        initial=0.0, op0=OP.mult, op1=OP.add)
                    p2 = s3.tile([128, 2, T], BF16, tag="p2")
                    pe = nc.vector
                    pe.tensor_tensor(out=p2.rearrange("p a b -> p (a b)"),
                                     in0=h2.rearrange("p a b -> p (a b)"),
                                     in1=cr.rearrange("p a b -> p (a b)"), op=OP.mult)
                    nl1 = s3a.tile([128, T], BF16, tag=f"l1_{k % 2}")
                    nc.vector.tensor_tensor(out=nl1, in0=p2[:, 0, :], in1=p2[:, 1, :],
                                            op=OP.add)
                    if k % 2 == 0:
                        l1 = nl1
                    else:
                        nl2 = s3a.tile([128, T], BF16, tag=f"l2_{(k // 2) % 2}")
                        nc.vector.tensor_tensor(out=nl2, in0=l1, in1=nl1, op=OP.add)
                        if k % 4 == 1:
                            l2 = nl2
                        else:
                            nl3 = s3a.tile([128, T], BF16, tag=f"l3_{k // 4}")
                            nc.vector.tensor_tensor(out=nl3, in0=l2, in1=nl2, op=OP.add)
                            if k == 3:
                                l3a = nl3
                            else:
                                l3b = nl3
                ysum = s3a.tile([128, T], BF16, tag="ysum")
                nc.vector.tensor_tensor(out=ysum, in0=l3a, in1=l3b, op=OP.add)
                ytot = s3a.tile([128, T], BF16, tag="ytot")
                nc.vector.scalar_tensor_tensor(out=ytot, in0=u_j,
                                               scalar=dv_t[:, j:j + 1], in1=ysum,
                                               op0=OP.mult, op1=OP.add)
                nc.vector.tensor_tensor(out=g_t[:, j, :], in0=ytot,
                                        in1=sz_j, op=OP.mult)
                nc.sync.dma_start(out=gsend[j][:], in_=g_t[:, j, :])
                nc.gpsimd.collective_compute(
                    "AllGather", OP.bypass,
                    replica_groups=[[0, 1], [2, 3], [4, 5], [6, 7]],
                    ins=[gsend[j][:].opt()],
                    outs=[ggath[j][:].opt()],
                )

        # ================= Phase 4: P matmul + GLU + GroupNorm =================
        with tc.tile_pool(name="s4", bufs=2) as s4, \
             tc.tile_pool(name="s4g", bufs=1) as s4g, \
             tc.tile_pool(name="ps4", bufs=2, space="PSUM") as ps4:
            goth = s4g.tile([128, DB, T], BF16)
            for j in range(DB):
                nc.sync.dma_start(out=goth[:, j, :], in_=ggath[j][1, :, :])
            yg = s4g.tile([128, CB, T], F32)        # GLU result
            stats = s4g.tile([128, CB * TF, 6], F32)
            for po in range(CB):                    # pair (po, po+4)
                lhs = {}
                for (nm, src, col) in (("oa", mownT, po), ("ta", mothT, po),
                                       ("os", mownT, po + 4), ("tss", mothT, po + 4)):
                    tiles = []
                    for jj in range(DB):
                        lt = s4.tile([128, 128], BF16, tag=f"{nm}{jj}")
                        nc.sync.dma_start(out=lt, in_=src[ts(jj, 128), ts(col, 128)])
                        tiles.append(lt)
                    lhs[nm] = tiles
                for tf in range(TF):
                    ps_a = ps4.tile([128, 512], F32, tag="ps_a")
                    ps_s = ps4.tile([128, 512], F32, tag="ps_s")
                    for jj in range(DB):
                        nc.tensor.matmul(out=ps_a, lhsT=lhs["oa"][jj],
                                         rhs=g_t[:, jj, ts(tf, 512)],
                                         start=(jj == 0), stop=False)
                        nc.tensor.matmul(out=ps_s, lhsT=lhs["os"][jj],
                                         rhs=g_t[:, jj, ts(tf, 512)],
                                         start=(jj == 0), stop=False)
                    for jj in range(DB):
                        nc.tensor.matmul(out=ps_a, lhsT=lhs["ta"][jj],
                                         rhs=goth[:, jj, ts(tf, 512)],
                                         start=False, stop=(jj == DB - 1))
                        nc.tensor.matmul(out=ps_s, lhsT=lhs["tss"][jj],
                                         rhs=goth[:, jj, ts(tf, 512)],
                                         start=False, stop=(jj == DB - 1))
                    ya = s4.tile([128, 512], F32, tag="ya")
                    sig = s4.tile([128, 512], F32, tag="sig")
                    nc.scalar.activation(out=ya, in_=ps_a, func=AF.Identity,
                                         bias=cvb_t[:, po:po + 1], scale=1.0)
                    nc.scalar.activation(out=sig, in_=ps_s, func=AF.Sigmoid,
                                         bias=cvb_t[:, po + 4:po + 5], scale=1.0)
                    nc.vector.tensor_tensor(out=yg[:, po, ts(tf, 512)],
                                            in0=ya, in1=sig, op=OP.mult)
                    nc.vector.bn_stats(out=stats[:, po * TF + tf, :],
                                       in_=yg[:, po, ts(tf, 512)])
            # GroupNorm over all (C, T)
            mv = s4.tile([128, 2], F32, tag="mv")
            nc.vector.bn_aggr(out=mv, in_=stats)
            stat2 = s4.tile([128, 2], F32, tag="stat2")
            nc.vector.tensor_copy(out=stat2[:, 0:1], in_=mv[:, 0:1])
            nc.vector.scalar_tensor_tensor(out=stat2[:, 1:2], in0=mv[:, 0:1],
                                           scalar=mv[:, 0:1], in1=mv[:, 1:2],
                                           op0=OP.mult, op1=OP.add)  # m^2 + v
            ps_st = ps4.tile([2, 1], F32, tag="ps_st")
            nc.tensor.matmul(out=ps_st, lhsT=stat2, rhs=ones_t,
                             start=True, stop=True)
            sc_sb = s4.tile([2, 1], F32, tag="sc_sb")
            nc.scalar.copy(out=sc_sb, in_=ps_st)
            nc.sync.dma_start(out=st_dram[0, :], in_=sc_sb[:, 0:1])
            st_bc = s4.tile([128, 2], F32, tag="st_bc")
            nc.sync.dma_start(out=st_bc, in_=_bcast_row(st_dram[0, :]))
            mu = s4.tile([128, 1], F32, tag="mu")
            e2 = s4.tile([128, 1], F32, tag="e2")
            nc.vector.tensor_scalar(out=mu, in0=st_bc[:, 0:1], scalar1=1.0 / 128,
                                    scalar2=None, op0=OP.mult)
            nc.vector.tensor_scalar(out=e2, in0=st_bc[:, 1:2], scalar1=1.0 / 128,
                                    scalar2=None, op0=OP.mult)
            mu2 = s4.tile([128, 1], F32, tag="mu2")
            nc.vector.tensor_tensor(out=mu2, in0=mu, in1=mu, op=OP.mult)
            var = s4.tile([128, 1], F32, tag="var")
            nc.vector.tensor_tensor(out=var, in0=e2, in1=mu2, op=OP.subtract)
            sdev = s4.tile([128, 1], F32, tag="sdev")
            nc.scalar.activation(out=sdev, in_=var, func=AF.Sqrt,
                                 bias=eps_t, scale=1.0)
            rstd = s4.tile([128, 1], F32, tag="rstd")
            nc.vector.reciprocal(out=rstd, in_=sdev)
            s1c = s4.tile([128, CB], F32, tag="s1c")
            nc.vector.tensor_scalar(out=s1c, in0=gnw_t, scalar1=rstd,
                                    scalar2=None, op0=OP.mult)
            nmu = s4.tile([128, 1], F32, tag="nmu")
            nc.vector.tensor_scalar(out=nmu, in0=mu, scalar1=-1.0,
                                    scalar2=None, op0=OP.mult)
            s2c = s4.tile([128, CB], F32, tag="s2c")
            nc.vector.scalar_tensor_tensor(out=s2c, in0=s1c, scalar=nmu,
                                           in1=gnb_t, op0=OP.mult, op1=OP.add)
            for cb in range(CB):
                o_t = s4.tile([128, T], F32, tag="o_t")
                nc.vector.tensor_scalar(out=o_t, in0=yg[:, cb, :],
                                        scalar1=s1c[:, cb:cb + 1],
                                        scalar2=s2c[:, cb:cb + 1],
                                        op0=OP.mult, op1=OP.add)
                nc.sync.dma_start(out=out[ts(cb, 128), :], in_=o_t)

    nc.finalize()
    return nc


def _prep_core_inputs(inputs):
    """Host-side weight prep -> list of 8 per-core input dicts."""
    f32 = np.float32
    bf16 = ml_dtypes.bfloat16
    x = np.asarray(inputs["x"], f32)
    conv_w = np.asarray(inputs["conv_w"], f32)
    conv_b = np.asarray(inputs["conv_b"], f32)
    gn_w = np.asarray(inputs["gn_w"], f32)
    gn_b = np.asarray(inputs["gn_b"], f32)

    def dirpack(p):
        Win = np.asarray(inputs[p + "Win"], f32)
        convw = np.asarray(inputs[p + "convw"], f32)
        convbv = np.asarray(inputs[p + "convb"], f32)
        Wx = np.asarray(inputs[p + "Wx"], f32)
        Wdt = np.asarray(inputs[p + "Wdt"], f32)
        bdtv = np.asarray(inputs[p + "bdt"], f32)
        Alog = np.asarray(inputs[p + "Alog"], f32)
        Dv = np.asarray(inputs[p + "Dv"], f32)
        Wout = np.asarray(inputs[p + "Wout"], f32)
        negA = np.exp(Alog)   # dA = exp(lnS * negA), lnS = -dt
        return {
            "winT": np.ascontiguousarray(Win.T),
            "wxT": np.ascontiguousarray(Wx.T).astype(bf16),
            "wdtT": np.ascontiguousarray(Wdt.T),
            "a_sc": np.ascontiguousarray(negA.reshape(DB, 128, N).transpose(1, 0, 2)),
            "taps": np.ascontiguousarray(convw[:, 0, :].reshape(DB, 128, 4).transpose(1, 0, 2)),
            "convb": np.ascontiguousarray(convbv.reshape(DB, 128).T),
            "bdt": np.ascontiguousarray(-bdtv.reshape(DB, 128).T),
            "dv": np.ascontiguousarray(Dv.reshape(DB, 128).T),
            "Wout": Wout,
        }

    fw = dirpack("fw_")
    bw = dirpack("bw_")
    Mf = conv_w[:, :C, 0] @ fw.pop("Wout")
    Mb = conv_w[:, C:, 0] @ bw.pop("Wout")
    MfT = np.ascontiguousarray(Mf.T).astype(bf16)
    MbT = np.ascontiguousarray(Mb.T).astype(bf16)

    shared = {
        "cvb": np.ascontiguousarray(conv_b.reshape(DB, 128).T),
        "gnw": np.ascontiguousarray(gn_w.reshape(CB, 128).T),
        "gnb": np.ascontiguousarray(gn_b.reshape(CB, 128).T),
    }
    in_maps = []
    for b in range(4):
        for d in range(2):
            if d == 0:
                m = dict(fw)
                m["x"] = np.ascontiguousarray(x[b])
                m["mownT"] = MfT
                m["mothT"] = MbT
            else:
                m = dict(bw)
                m["x"] = np.ascontiguousarray(x[b][:, ::-1])
                m["mownT"] = MbT
                m["mothT"] = MfT
            m.update(shared)
            in_maps.append(m)
    return in_maps


def kernel(**inputs):
    if "nc" not in _CACHE:
        _CACHE["nc"] = build_program()
    nc = _CACHE["nc"]
    in_maps = _prep_core_inputs(inputs)
    res = run_bass_kernel_spmd(nc, in_maps, list(range(8)))
    out = np.stack([np.asarray(res.results[2 * b]["out"]) for b in range(4)])
    return out.astype(np.float32)


if __name__ == "__main__":
    import reference
    inputs = {k: np.asarray(v) for k, v in reference.setup_inputs().items()}
    got = kernel(**inputs)
    print("out", got.shape, got.dtype)
